# revision 39
# baseline (speedup 1.0000x reference)
"""EnsRec loss kernel for 8 Trainium2 NeuronCores.

Data-parallel over batch (64 rows per core); item/user tables and W_proj
replicated. Algebraic restructuring vs the reference:
  basemodel_emb = (sum_l tw[l]*mask*E[ids]) @ W_proj + b_proj*sum(tw)
(projection commutes with the time-decay sum, and the validity mask is
folded into the per-hit weight so id==0 rows need no table edit).

v5 design:
- Item table and one-hot scatter weights are fp8-e4m3 (table pre-scaled
  by 64 on the host; the 1/64 is folded into W_proj in fp32), halving
  gather DMA bytes and enabling DoubleRow matmuls.
- The 25600 per-core table-row gathers are issued as one batched
  `dma_gather` per (bk-chunk, 32768-row table range) — ids are
  range-split so the int16 index constraint holds — spread round-robin
  over 4 SWDGE queues so descriptor generation runs on all four Q7
  core-pairs in parallel.
- The weighted accumulate runs on the TensorEngine as one-hot scatter
  matmuls (acc += S_j^T @ G_j in PSUM), two gather columns per matmul
  via perf_mode=DoubleRow. S tiles are precomputed on the host and
  streamed in via HWDGE.
- PSUM->SBUF copies ride the otherwise-idle Scalar engine.

Each core emits per-row partial losses; the host does the final 8-way sum.
"""

import sys

import numpy as np

_TRN_REPO = "/opt/trn_rl_repo"
if _TRN_REPO not in sys.path:
    sys.path.insert(0, _TRN_REPO)

import concourse.bacc as bacc
import concourse.mybir as mybir
import concourse.tile as tile
from concourse.bass import IndirectOffsetOnAxis
from concourse.bass_utils import run_bass_kernel_spmd

B, K, L, D, H = 512, 8, 50, 768, 128
N_ITEM = 100000
N_USER = 50000
DIV_TRADEOFF = 0.1
NCORES = 8
BLOC = B // NCORES          # 64 batch rows per core
BK = BLOC * K               # 512 (b,k) rows per core
NCHUNK = BK // 128          # 4 partition-tiles of (b,k) rows
RB = 15                     # table split into 2**RB-row ranges for int16 idx
NRANGE = (N_ITEM >> RB) + 1
NQ = 4                      # SWDGE queues for gather descgen parallelism
TSCALE = 64.0               # host table scale (compensated via W_proj)
USE_DR = True               # DoubleRow paired scatter matmuls
OUT_LEN = 2 * BLOC

_f32 = mybir.dt.float32
_bf16 = mybir.dt.bfloat16
_fp8 = mybir.dt.float8e4
_i32 = mybir.dt.int32
_i16 = mybir.dt.int16
ALU = mybir.AluOpType
AFT = mybir.ActivationFunctionType
AXL = mybir.AxisListType
DR = mybir.MatmulPerfMode.DoubleRow

_CACHED = {}


def _build_module(cols, tot_cols, gcap):
    """cols[t][m] = gather columns (128 hits each) for chunk t, range m."""
    nc = bacc.Bacc("TRN2", target_bir_lowering=False, debug=False,
                   num_devices=NCORES, num_swdge_queues=NQ)

    table = nc.dram_tensor("table", [N_ITEM + 1, D], _fp8, kind="ExternalInput")
    utable = nc.dram_tensor("utable", [N_USER, H], _f32, kind="ExternalInput")
    wproj = nc.dram_tensor("wproj", [D, H], _f32, kind="ExternalInput")
    beff = nc.dram_tensor("beff", [H, 1], _f32, kind="ExternalInput")
    bmask = nc.dram_tensor("bmask", [128, 128], _f32, kind="ExternalInput")
    identin = nc.dram_tensor("identin", [128, 128], _f32, kind="ExternalInput")
    eind = nc.dram_tensor("eind", [128, 4 * NCHUNK], _f32, kind="ExternalInput")
    smat = nc.dram_tensor("smat", [128, tot_cols * 128], _fp8,
                          kind="ExternalInput")
    idx16 = nc.dram_tensor("idx16", [128, tot_cols * 8], _i16,
                           kind="ExternalInput")
    uid = nc.dram_tensor("uid", [BLOC, 1], _i32, kind="ExternalInput")
    prefin = nc.dram_tensor("prefin", [BLOC, H], _f32, kind="ExternalInput")
    posT = nc.dram_tensor("posT", [NCHUNK, 128], _f32, kind="ExternalInput")
    negT = nc.dram_tensor("negT", [NCHUNK, 128], _f32, kind="ExternalInput")
    out = nc.dram_tensor("out", [OUT_LEN], _f32, kind="ExternalOutput")
    wscr1 = nc.dram_tensor("wscr1", [BK], _f32)

    with tile.TileContext(nc) as tc:
        with (
            tc.tile_pool(name="gp", bufs=14) as gp,
            tc.tile_pool(name="sb", bufs=1) as sbp,
            tc.tile_pool(name="work", bufs=2) as workp,
            tc.tile_pool(name="pacc", bufs=2, space="PSUM") as pacc,
            tc.tile_pool(name="ps2", bufs=2, space="PSUM") as ps2,
            tc.tile_pool(name="ps1", bufs=1, space="PSUM") as ps1,
        ):
            # warm-up gather: absorbs the Q7 ext-isa IRAM load (~6us) while
            # the real index data is still streaming in
            widx = sbp.tile([128, 8], _i16, tag="widx")
            nc.vector.memset(widx[:], 0)
            wgt = sbp.tile([128, D], _fp8, tag="wgt")
            nc.gpsimd.dma_gather(
                out_ap=wgt[:].rearrange("p (j d) -> p j d", d=D),
                in_ap=table[0:128], idxs_ap=widx[:],
                num_idxs=128, num_idxs_reg=128, elem_size=D,
                single_packet=False, queue_num=0)
            # gather-critical loads first
            idx_sb = sbp.tile([128, tot_cols * 8], _i16, tag="idx")
            nc.sync.dma_start(out=idx_sb[:], in_=idx16[:])
            smat_sb = sbp.tile([128, tot_cols * 128], _fp8, tag="smat")
            ident = sbp.tile([128, 128], _f32, tag="ident")
            nc.sync.dma_start(out=ident[:], in_=identin[:])
            wall = sbp.tile([128, 6 * 128], _f32, tag="wall")
            for c in range(6):
                nc.sync.dma_start(out=wall[:, c * 128:(c + 1) * 128],
                                  in_=wproj[c * 128:(c + 1) * 128, :])
            beff_sb = sbp.tile([H, 1], _f32, tag="beff")
            nc.sync.dma_start(out=beff_sb[:], in_=beff[:])
            bmask_sb = sbp.tile([128, 128], _f32, tag="bmask")
            nc.sync.dma_start(out=bmask_sb[:], in_=bmask[:])
            eind_sb = sbp.tile([128, 4 * NCHUNK], _f32, tag="eind")
            nc.sync.dma_start(out=eind_sb[:], in_=eind[:])
            uid_sb = sbp.tile([BLOC, 1], _i32, tag="uid")
            nc.sync.dma_start(out=uid_sb[:], in_=uid[:])
            prefin_sb = sbp.tile([BLOC, H], _f32, tag="prefin")
            nc.sync.dma_start(out=prefin_sb[:], in_=prefin[:])

            wsumT = sbp.tile([128, 6 * 512], _f32, tag="wsumT")
            eT = sbp.tile([128, 512], _f32, tag="eT")
            r_all = sbp.tile([128, NCHUNK], _f32, tag="rall")
            ones = sbp.tile([128, 1], _f32, tag="ones")
            nc.vector.memset(ones[:], 1.0)
            pref = sbp.tile([BLOC, H], _f32, tag="pref")
            prep = sbp.tile([128, 512], _f32, tag="prep")
            wop4 = ps1.tile([NCHUNK, 128], _f32, tag="wop")

            # ---- main gather + PE scatter-accumulate ----
            # dma_gather writes hit i of a group to [i%128, i//128, :]; the
            # host-built S_j[p, r] = w(hit) * [r == target row of hit] tiles
            # stream in via HWDGE, and the TensorEngine accumulates
            # acc += S_j^T @ G_j in PSUM, two columns per DoubleRow matmul.
            smat3 = smat_sb[:].rearrange("p (j r) -> p j r", r=128)
            coff = 0
            qi = 0
            for t in range(NCHUNK):
                accA = pacc.tile([128, 384], _f32, tag="accA")
                accB = pacc.tile([128, 384], _f32, tag="accB")
                tcols = sum(cols[t])
                # stream this chunk's S tiles (HWDGE, overlaps the gathers)
                nc.sync.dma_start(
                    out=smat_sb[:, coff * 128:(coff + tcols) * 128],
                    in_=smat[:, coff * 128:(coff + tcols) * 128])
                jg = 0
                mm_ops = []          # (smat col, gt3, jl) pending matmul cols
                for m in range(NRANGE):
                    gcols = cols[t][m]
                    if gcols == 0:
                        continue
                    rbase = m << RB
                    rlen = min(N_ITEM + 1 - rbase, 1 << RB)
                    # split big groups: finer queue/ring interleave, and the
                    # chunk's matmuls start as soon as the first half lands
                    half = (gcols + 1) // 2 // 2 * 2 if gcols >= 10 else gcols
                    for (a, b) in ([(0, half), (half, gcols)]
                                   if half < gcols else [(0, gcols)]):
                        scols = b - a
                        gt = gp.tile([128, ((gcap + 1) // 2 + 1) * D], _fp8,
                                     tag="gath")
                        gt3 = gt[:].rearrange("p (j d) -> p j d", d=D)
                        nc.gpsimd.dma_gather(
                            out_ap=gt3[:, 0:scols, :],
                            in_ap=table[rbase:rbase + rlen],
                            idxs_ap=idx_sb[:, (coff + jg + a) * 8:
                                           (coff + jg + b) * 8],
                            num_idxs=128 * scols,
                            num_idxs_reg=128 * scols,
                            elem_size=D,
                            single_packet=False,
                            queue_num=qi % NQ,
                        )
                        qi += 1
                        for jl in range(scols):
                            mm_ops.append((coff + jg + a + jl, gt3, jl))
                    jg += gcols
                # emit scatter matmuls; pair columns from the same gather tile
                first = True
                i = 0
                n_ops = len(mm_ops)
                while i < n_ops:
                    sc, g3, jl = mm_ops[i]
                    pairable = (USE_DR and i + 1 < n_ops
                                and mm_ops[i + 1][1] is g3
                                and mm_ops[i + 1][2] == jl + 1)
                    last = (i + (2 if pairable else 1)) >= n_ops
                    if pairable:
                        sjp = smat3[:, sc:sc + 2, :]
                        nc.tensor.matmul(out=accA[:], lhsT=sjp,
                                         rhs=g3[:, jl:jl + 2, 0:384],
                                         start=first, stop=last, perf_mode=DR)
                        nc.tensor.matmul(out=accB[:], lhsT=sjp,
                                         rhs=g3[:, jl:jl + 2, 384:768],
                                         start=first, stop=last, perf_mode=DR)
                        i += 2
                    else:
                        sj = smat3[:, sc, :]
                        nc.tensor.matmul(out=accA[:], lhsT=sj,
                                         rhs=g3[:, jl, 0:384],
                                         start=first, stop=last)
                        nc.tensor.matmul(out=accB[:], lhsT=sj,
                                         rhs=g3[:, jl, 384:768],
                                         start=first, stop=last)
                        i += 1
                    first = False
                coff += tcols
                if t == 0:
                    # preference = prefin + utable[uid]; off the critical path
                    nc.gpsimd.indirect_dma_start(
                        out=pref[:], out_offset=None, in_=utable[:],
                        in_offset=IndirectOffsetOnAxis(ap=uid_sb[:, :1], axis=0))
                    nc.vector.tensor_tensor(out=pref[:], in0=pref[:],
                                            in1=prefin_sb[:], op=ALU.add)
                    ptp = ps1.tile([128, BLOC], _f32, tag="ptp")
                    nc.tensor.transpose(out=ptp[:], in_=pref[:],
                                        identity=ident[:BLOC, :BLOC])
                    prep3 = prep[:].rearrange("p (b k) -> p b k", k=K)
                    for k in range(K):
                        nc.vector.tensor_copy(out=prep3[:, :, k], in_=ptp[:])
                acc = workp.tile([128, D], _f32, tag="acc")
                nc.scalar.activation(out=acc[:, 0:384], in_=accA[:],
                                     func=AFT.Copy)
                nc.scalar.activation(out=acc[:, 384:768], in_=accB[:],
                                     func=AFT.Copy)
                # per-chunk tail: transpose, project, gram, score
                for c in range(6):
                    tp = ps2.tile([128, 128], _f32, tag="tp")
                    nc.tensor.transpose(out=tp[:],
                                        in_=acc[:, c * 128:(c + 1) * 128],
                                        identity=ident[:])
                    nc.scalar.activation(
                        out=wsumT[:, c * 512 + t * 128: c * 512 + (t + 1) * 128],
                        in_=tp[:], func=AFT.Copy)
                eTp = ps2.tile([128, 128], _f32, tag="tp")
                for c in range(6):
                    nc.tensor.matmul(
                        out=eTp[:],
                        lhsT=wall[:, c * 128:(c + 1) * 128],
                        rhs=wsumT[:, c * 512 + t * 128: c * 512 + (t + 1) * 128],
                        start=(c == 0), stop=(c == 5))
                nc.vector.tensor_scalar(out=eT[:, t * 128:(t + 1) * 128],
                                        in0=eTp[:], scalar1=beff_sb[:],
                                        scalar2=None, op0=ALU.add)
                sp = ps2.tile([128, 128], _f32, tag="tp")
                nc.tensor.matmul(out=sp[:], lhsT=eT[:, t * 128:(t + 1) * 128],
                                 rhs=eT[:, t * 128:(t + 1) * 128],
                                 start=True, stop=True)
                spc = workp.tile([128, 128], _f32, tag="spc")
                nc.vector.tensor_copy(out=spc[:], in_=sp[:])
                s2 = workp.tile([128, 128], _f32, tag="s2")
                nc.vector.tensor_tensor(out=s2[:], in0=sp[:], in1=spc[:],
                                        op=ALU.mult)
                dummy = workp.tile([128, 128], _f32, tag="dummy")
                nc.vector.scalar_tensor_tensor(
                    out=dummy[:], in0=s2[:], scalar=1.0, in1=bmask_sb[:],
                    op0=ALU.mult, op1=ALU.mult, accum_out=r_all[:, t:t + 1])
                prod = workp.tile([128, 128], _f32, tag="prod")
                nc.vector.tensor_tensor(out=prod[:],
                                        in0=eT[:, t * 128:(t + 1) * 128],
                                        in1=prep[:, t * 128:(t + 1) * 128],
                                        op=ALU.mult)
                nc.tensor.matmul(out=wop4[:],
                                 lhsT=eind_sb[:, t * NCHUNK:(t + 1) * NCHUNK],
                                 rhs=prod[:],
                                 start=(t == 0), stop=(t == NCHUNK - 1))

            # ---- tail in [NCHUNK, 128] layout: partition t holds bk-rows
            # t*128..t*128+127, so k-groups (8) and b-groups (16/partition)
            # stay within a partition. u = exp(worg) (no max-sub: |worg| <~
            # 6); softmax normalization is folded into the pu/nu and div
            # ratios, so no k-broadcast is needed.
            u4 = sbp.tile([NCHUNK, 128], _f32, tag="u4")
            nc.scalar.activation(out=u4[:], in_=wop4[:], func=AFT.Exp)
            u43 = u4[:].rearrange("t (b k) -> t b k", k=K)
            s4 = sbp.tile([NCHUNK, 16], _f32, tag="s4")
            nc.vector.tensor_reduce(out=s4[:], in_=u43, axis=AXL.X, op=ALU.add)
            rs4 = sbp.tile([NCHUNK, 16], _f32, tag="rs4")
            nc.vector.reciprocal(out=rs4[:], in_=s4[:])

            pos_sb = sbp.tile([NCHUNK, 128], _f32, tag="pos")
            nc.sync.dma_start(out=pos_sb[:], in_=posT[:])
            neg_sb = sbp.tile([NCHUNK, 128], _f32, tag="neg")
            nc.sync.dma_start(out=neg_sb[:], in_=negT[:])
            pn = sbp.tile([NCHUNK, 128], _f32, tag="pn")
            nc.vector.tensor_tensor(out=pn[:], in0=pos_sb[:], in1=neg_sb[:],
                                    op=ALU.subtract)
            pnu = sbp.tile([NCHUNK, 128], _f32, tag="pnu")
            nc.vector.tensor_tensor(out=pnu[:], in0=pn[:], in1=u4[:],
                                    op=ALU.mult)
            dnum = sbp.tile([NCHUNK, 16], _f32, tag="dnum")
            nc.vector.tensor_reduce(
                out=dnum[:], in_=pnu[:].rearrange("t (b k) -> t b k", k=K),
                axis=AXL.X, op=ALU.add)
            dlt = sbp.tile([NCHUNK, 16], _f32, tag="dlt")
            nc.vector.tensor_tensor(out=dlt[:], in0=dnum[:], in1=rs4[:],
                                    op=ALU.mult)
            expt = sbp.tile([NCHUNK, 16], _f32, tag="expt")
            nc.scalar.activation(out=expt[:], in_=dlt[:], func=AFT.Exp,
                                 scale=-1.0)
            bce = sbp.tile([NCHUNK, 16], _f32, tag="bce")
            nc.scalar.activation(out=bce[:], in_=expt[:], func=AFT.Ln,
                                 bias=1.0)
            nc.sync.dma_start(
                out=out[0:BLOC].rearrange("(t f) -> t f", f=16), in_=bce[:])

            # ---- div part: rts is already [NCHUNK, 128];
            # out[64:128] = per-b sum_k u*r / s ----
            rtp = ps1.tile([NCHUNK, 128], _f32, tag="ptp")
            nc.tensor.transpose(out=rtp[:], in_=r_all[:], identity=ident[:])
            ur = sbp.tile([NCHUNK, 128], _f32, tag="ur")
            nc.vector.tensor_tensor(out=ur[:], in0=u4[:], in1=rtp[:],
                                    op=ALU.mult)
            urg = sbp.tile([NCHUNK, 16], _f32, tag="urg")
            nc.vector.tensor_reduce(
                out=urg[:], in_=ur[:].rearrange("t (b k) -> t b k", k=K),
                axis=AXL.X, op=ALU.add)
            dvb = sbp.tile([NCHUNK, 16], _f32, tag="dvb")
            nc.vector.tensor_tensor(out=dvb[:], in0=urg[:], in1=rs4[:],
                                    op=ALU.mult)
            nc.sync.dma_start(
                out=out[BLOC:OUT_LEN].rearrange("(t f) -> t f", f=16),
                in_=dvb[:])

    nc.compile()
    return nc


def _get_nc(cols, tot_cols, gcap):
    key = (tuple(map(tuple, cols)), tot_cols, gcap)
    if key not in _CACHED:
        _CACHED[key] = _build_module(cols, tot_cols, gcap)
    return _CACHED[key]


def _prep_in_maps(user_id, base_model_preds, preference_in, pos_label,
                  neg_label, user_embeddings, item_embeddings, W_proj, b_proj):
    tw = (1.0 / np.log2(np.arange(L, dtype=np.float32) + 2.0)).astype(np.float32)
    import ml_dtypes
    fp8 = ml_dtypes.float8_e4m3
    table = np.ascontiguousarray(
        (np.asarray(item_embeddings, dtype=np.float32) * TSCALE).astype(fp8))
    utable = np.ascontiguousarray(np.asarray(user_embeddings, dtype=np.float32))
    wproj = np.ascontiguousarray(
        np.asarray(W_proj, dtype=np.float32) / TSCALE)
    beff = (np.asarray(b_proj, dtype=np.float32) * np.float32(tw.sum())
            ).reshape(H, 1)
    ident_np = np.eye(128, dtype=np.float32)
    eind_np = np.zeros((128, 4 * NCHUNK), dtype=np.float32)
    for t in range(NCHUNK):
        eind_np[:, t * NCHUNK + t] = 1.0
    bmask = (np.kron(np.eye(16, dtype=np.float32),
                     np.ones((8, 8), dtype=np.float32))
             - np.eye(128, dtype=np.float32)).astype(np.float32)

    preds = np.asarray(base_model_preds).astype(np.int64)
    uid_all = np.asarray(user_id).astype(np.int32).reshape(B, 1)
    pref_all = np.asarray(preference_in, dtype=np.float32)
    pos_all = np.asarray(pos_label, dtype=np.float32)
    neg_all = np.asarray(neg_label, dtype=np.float32)

    # ---- per (core, chunk, range): range-sorted padded hit groups ----
    # hit = (target row r in chunk, l); groups padded to 128-multiples with
    # (local_id=0, r=0, w=0) so every gathered slot holds finite table data.
    per_core = []
    raw_cols = np.zeros((NCORES, NCHUNK, NRANGE), dtype=np.int64)
    for c in range(NCORES):
        s = slice(c * BLOC, (c + 1) * BLOC)
        pf = preds[s].reshape(BK, L)
        valid = (pf > 0) & (pf <= N_ITEM)
        safe = np.where(valid, pf, 0).astype(np.int64)
        wfull = tw[None, :] * valid.astype(np.float32)
        chunks = []
        for t in range(NCHUNK):
            ids = safe[t * 128:(t + 1) * 128].reshape(-1)     # r-major flat
            ws = wfull[t * 128:(t + 1) * 128].reshape(-1)
            rr = np.repeat(np.arange(128), L)
            m = ids >> RB
            groups = []
            for mm in range(NRANGE):
                sel = np.nonzero(m == mm)[0]
                raw_cols[c, t, mm] = (len(sel) + 127) // 128
                groups.append((ids[sel] - (mm << RB), rr[sel], ws[sel]))
            chunks.append(groups)
        per_core.append(chunks)
    # uniform column counts across cores (one shared SPMD module)
    cols = tuple(tuple(int(raw_cols[:, t, mm].max()) for mm in range(NRANGE))
                 for t in range(NCHUNK))
    tot_cols = int(sum(sum(ct) for ct in cols))
    gcap = int(max(max(ct) for ct in cols))

    in_maps = []
    for c in range(NCORES):
        idx16_np = np.zeros((128, tot_cols * 8), dtype=np.int16)
        smat_np = np.zeros((128, tot_cols, 128), dtype=np.float32)
        coff = 0
        for t in range(NCHUNK):
            for mm in range(NRANGE):
                gcols = cols[t][mm]
                if gcols == 0:
                    continue
                lids, rrs, wss = per_core[c][t][mm]
                n = 128 * gcols
                lid_p = np.zeros(n, dtype=np.int16)
                r_p = np.zeros(n, dtype=np.int64)
                w_p = np.zeros(n, dtype=np.float32)
                lid_p[:len(lids)] = lids
                r_p[:len(lids)] = rrs
                w_p[:len(lids)] = wss
                # idxs wrapped: hit i at [i%16, i//16], replicated to 128 rows
                iw = lid_p.reshape(gcols * 8, 16).T            # [16, 8*gcols]
                idx16_np[:, coff * 8:(coff + gcols) * 8] = np.tile(iw, (8, 1))
                # gathered layout: hit i -> [i%128, i//128]; S[p, j, r] = w
                rg = r_p.reshape(gcols, 128).T                 # [128, gcols]
                wg = w_p.reshape(gcols, 128).T
                blk = smat_np[:, coff:coff + gcols, :]
                np.put_along_axis(blk, rg[:, :, None], wg[:, :, None], axis=2)
                coff += gcols
        smat_f8 = np.ascontiguousarray(
            smat_np.reshape(128, tot_cols * 128).astype(fp8))
        s = slice(c * BLOC, (c + 1) * BLOC)
        in_maps.append({
            "table": table,
            "identin": ident_np,
            "eind": eind_np,
            "utable": utable,
            "wproj": wproj,
            "beff": beff,
            "bmask": bmask,
            "idx16": idx16_np,
            "smat": smat_f8,
            "uid": np.ascontiguousarray(uid_all[s]),
            "prefin": np.ascontiguousarray(pref_all[s]),
            "posT": np.ascontiguousarray(pos_all[s].reshape(NCHUNK, 128)),
            "negT": np.ascontiguousarray(neg_all[s].reshape(NCHUNK, 128)),
        })
    return in_maps, cols, tot_cols, gcap


def _reduce_outputs(results):
    bce_total = 0.0
    div_total = 0.0
    for r in results:
        o = np.asarray(r["out"], dtype=np.float64)
        bce_total += o[:BLOC].sum()
        div_total += o[BLOC:].sum()
    loss = bce_total + DIV_TRADEOFF * (2.0 * div_total) / (B * K * K)
    return np.asarray(loss, dtype=np.float32)


def prepare(inputs):
    in_maps, cols, tot_cols, gcap = _prep_in_maps(**inputs)
    nc = _get_nc(cols, tot_cols, gcap)
    return nc, in_maps


def kernel(**inputs):
    nc, in_maps = prepare(inputs)
    res = run_bass_kernel_spmd(nc, in_maps, list(range(NCORES)))
    return _reduce_outputs(res.results)
